# revision 6
# baseline (speedup 1.0000x reference)
"""SupCon cluster-memory loss kernel for 8 TRN2 NeuronCores.

Problem: 4 SupCon losses (rgb/ir anchors x rgb/ir memory banks).
  logits = l2norm(x) @ mem.T / T   [256, 8192]
  loss   = -mean_i[ (sum_j mask*logprob) / max(sum_j mask, 1) ]

Sharding: memory banks split column-wise (N=8192 -> 1024 per core),
anchor batches replicated.  Each core computes, for its N-shard and all
4 (anchor, bank) combos:
  - sumexp[i] = sum_j exp(logits_ij - shift_b)   (ScalarE Exp w/ accum_out)
  - pos[i]    = sum_j [lab_i == mlab_j] * logits_ij  (VectorE ttr)
Host combines shards: LSE = shift + log(sum_cores sumexp), positives and
match counts, then the 4 scalar losses.

shift_b = max_row_norm(bank_b)/T is a per-bank upper bound on |logits|
(anchors are unit-norm), so exp args are <= 0: no overflow, and with
unit-norm banks the dynamic range is e^-28..1 -- no harmful underflow.
"""

from contextlib import ExitStack

import numpy as np

import concourse.bacc as bacc
import concourse.bass as bass
import concourse.mybir as mybir
import concourse.tile as tile
from concourse.bass_utils import run_bass_kernel_spmd

B = 256          # anchor batch per modality
N = 8192         # memory bank rows
D = 768          # feature dim
NCORES = 8
NS = N // NCORES     # 1024 bank rows per core
KT = D // 128        # 6 contraction tiles
MT = B // 128        # 2 anchor partition tiles
NT = NS // 512       # 2 psum free-dim tiles
SUPCON_T = 0.07

F32 = mybir.dt.float32
F32R = mybir.dt.float32r

_NC_CACHE = {}


def _build_nc():
    nc = bacc.Bacc("TRN2", target_bir_lowering=False, debug=False,
                   num_devices=NCORES)

    # Per-core DRAM inputs (host pre-transposed to K-major layouts).
    xT = nc.dram_tensor("xT", [2, KT, 128, B], F32R, kind="ExternalInput").ap()
    memT = nc.dram_tensor("memT", [2, KT, 128, NS], F32R, kind="ExternalInput").ap()
    lab_h = nc.dram_tensor("lab", [2, B], F32, kind="ExternalInput")
    mlab_h = nc.dram_tensor("mlab", [2, NS], F32, kind="ExternalInput")
    nshift_h = nc.dram_tensor("nshift", [2], F32, kind="ExternalInput")
    # Outputs: per anchor-row partial sums.  col = mt*8 + c*2 + nt,
    # combo c = a*2 + b.
    res_s = nc.dram_tensor("res_s", [128, 16], F32, kind="ExternalOutput").ap()
    res_p = nc.dram_tensor("res_p", [128, 16], F32, kind="ExternalOutput").ap()

    with tile.TileContext(nc) as tc, ExitStack() as ctx:
        const = ctx.enter_context(tc.tile_pool(name="const", bufs=1))
        wpool = ctx.enter_context(tc.tile_pool(name="wpool", bufs=1))
        mpool = ctx.enter_context(tc.tile_pool(name="mpool", bufs=1))
        eqpool = ctx.enter_context(tc.tile_pool(name="eqpool", bufs=1))
        psum = ctx.enter_context(tc.tile_pool(name="psum", bufs=8, space="PSUM"))
        ep = ctx.enter_context(tc.tile_pool(name="ep", bufs=3))
        outp = ctx.enter_context(tc.tile_pool(name="outp", bufs=1))

        rs = outp.tile([128, 16], F32, tag="rs", name="rs")
        rp = outp.tile([128, 16], F32, tag="rp", name="rp")

        # Anchor labels as [p, mt] and the per-bank -shift scalars.
        labt = []
        for a in range(2):
            t = const.tile([128, MT], F32, tag=f"lab{a}", name=f"lab{a}")
            nc.sync.dma_start(out=t, in_=lab_h.ap()[a].rearrange("(mt p) -> p mt", p=128))
            labt.append(t)
        shift_t = const.tile([128, 2], F32, tag="shift", name="shift")
        nc.sync.dma_start(out=shift_t,
                          in_=bass.AP(tensor=nshift_h, offset=0, ap=[[0, 128], [1, 2]]))

        # Bank prototype labels broadcast across partitions.
        mlabt = []
        for b in range(2):
            t = const.tile([128, NS], F32, tag=f"mlab{b}", name=f"mlab{b}")
            nc.sync.dma_start(out=t,
                              in_=bass.AP(tensor=mlab_h, offset=b * NS,
                                          ap=[[0, 128], [1, NS]]))
            mlabt.append(t)

        # Anchor features, K-major: [128, B] per (a, kt).
        xt = [[wpool.tile([128, B], F32R, tag=f"x{a}_{k}", name=f"x{a}_{k}") for k in range(KT)]
              for a in range(2)]
        for a in range(2):
            for k in range(KT):
                nc.sync.dma_start(out=xt[a][k], in_=xT[a, k])

        # Bank shards, K-major: [128, NS] per (b, kt); kt-major issue order so
        # the accumulation loop's operands arrive first.
        memt = [[None] * KT for _ in range(2)]
        for k in range(KT):
            for b in range(2):
                t = mpool.tile([128, NS], F32R, tag=f"m{b}_{k}", name=f"m{b}_{k}")
                nc.sync.dma_start(out=t, in_=memT[b, k])
                memt[b][k] = t

        # Label-match masks (GpSimd, frees VectorE for the reductions).
        eqt = {}
        for a in range(2):
            for b in range(2):
                for mt in range(MT):
                    t = eqpool.tile([128, NS], F32, tag=f"eq{a}{b}{mt}", name=f"eq{a}{b}{mt}")
                    nc.gpsimd.tensor_scalar(out=t, in0=mlabt[b],
                                            scalar1=labt[a][:, mt:mt + 1],
                                            scalar2=None,
                                            op0=mybir.AluOpType.is_equal)
                    eqt[a, b, mt] = t

        for mt in range(MT):
            acc = {}
            for a in range(2):
                for b in range(2):
                    for nt in range(NT):
                        acc[a, b, nt] = psum.tile([128, 512], F32, tag="acc", name=f"acc{mt}_{a}{b}{nt}")
            for k in range(KT):
                for a in range(2):
                    lhsT = xt[a][k][:, mt * 128:(mt + 1) * 128]
                    for b in range(2):
                        for nt in range(NT):
                            nc.tensor.matmul(
                                acc[a, b, nt][:],
                                lhsT,
                                memt[b][k][:, nt * 512:(nt + 1) * 512],
                                start=(k == 0), stop=(k == KT - 1))
            for a in range(2):
                for b in range(2):
                    c = a * 2 + b
                    for nt in range(NT):
                        col = mt * 8 + c * 2 + nt
                        ex = ep.tile([128, 512], F32, tag="ex", name=f"ex{mt}_{c}{nt}")
                        nc.scalar.activation(
                            out=ex, in_=acc[a, b, nt][:],
                            func=mybir.ActivationFunctionType.Exp,
                            bias=shift_t[:, b:b + 1],
                            scale=1.0 / SUPCON_T,
                            accum_out=rs[:, col:col + 1])
                        # tensor_tensor_reduce faults on this runtime's DVE
                        # ucode; use mult + reduce instead.
                        msk = ep.tile([128, 512], F32, tag="msk", name=f"msk{mt}_{c}{nt}")
                        nc.vector.tensor_tensor(
                            out=msk,
                            in0=eqt[a, b, mt][:, nt * 512:(nt + 1) * 512],
                            in1=acc[a, b, nt][:],
                            op=mybir.AluOpType.mult)
                        nc.vector.tensor_reduce(
                            out=rp[:, col:col + 1], in_=msk,
                            axis=mybir.AxisListType.X,
                            op=mybir.AluOpType.add)

        nc.sync.dma_start(out=res_s, in_=rs)
        nc.sync.dma_start(out=res_p, in_=rp)

    nc.compile()
    return nc


def get_nc():
    if "nc" not in _NC_CACHE:
        _NC_CACHE["nc"] = _build_nc()
    return _NC_CACHE["nc"]


def _l2norm(x):
    n = np.linalg.norm(x, axis=-1, keepdims=True)
    return x / np.maximum(n, 1e-12)


def make_in_maps(inputs_rgb, inputs_ir, targets_rgb, targets_ir,
                 features_rgb, features_ir,
                 prototype_labels_rgb, prototype_labels_ir):
    x = [_l2norm(np.asarray(inputs_rgb, np.float32)),
         _l2norm(np.asarray(inputs_ir, np.float32))]
    feats = [np.asarray(features_rgb, np.float32),
             np.asarray(features_ir, np.float32)]
    lab = np.stack([np.asarray(targets_rgb), np.asarray(targets_ir)]
                   ).astype(np.float32)                       # [2, B]
    mlab_full = np.stack([np.asarray(prototype_labels_rgb),
                          np.asarray(prototype_labels_ir)]).astype(np.float32)

    xT = np.empty([2, KT, 128, B], np.float32)
    for a in range(2):
        xT[a] = x[a].T.reshape(KT, 128, B)

    bank_max = [float(np.sqrt((feats[b] ** 2).sum(axis=1).max()))
                for b in range(2)]
    shift = np.array([bank_max[0] / SUPCON_T, bank_max[1] / SUPCON_T],
                     np.float64)                              # added back on host
    nshift = (-shift).astype(np.float32)

    in_maps = []
    for c in range(NCORES):
        memT = np.empty([2, KT, 128, NS], np.float32)
        for b in range(2):
            memT[b] = np.ascontiguousarray(
                feats[b][c * NS:(c + 1) * NS, :].T).reshape(KT, 128, NS)
        in_maps.append({
            "xT": xT,
            "memT": memT,
            "lab": lab,
            "mlab": np.ascontiguousarray(mlab_full[:, c * NS:(c + 1) * NS]),
            "nshift": nshift,
        })
    return in_maps, shift


def combine(results, shift, targets_rgb, targets_ir,
            prototype_labels_rgb, prototype_labels_ir):
    # res arrays: [128, 16], col = mt*8 + c*2 + nt
    rs = np.stack([np.asarray(r["res_s"], np.float64) for r in results])
    rp = np.stack([np.asarray(r["res_p"], np.float64) for r in results])
    rs = rs.reshape(NCORES, 128, MT, 4, NT).sum(axis=(0, 4))  # [128, mt, c]
    rp = rp.reshape(NCORES, 128, MT, 4, NT).sum(axis=(0, 4))
    sumexp = rs.transpose(1, 0, 2).reshape(B, 4)              # anchor i = mt*128+p
    pos = rp.transpose(1, 0, 2).reshape(B, 4)

    lab = [np.asarray(targets_rgb).astype(np.int64),
           np.asarray(targets_ir).astype(np.int64)]
    mlab = [np.asarray(prototype_labels_rgb).astype(np.int64),
            np.asarray(prototype_labels_ir).astype(np.int64)]

    losses = np.zeros(4, np.float64)
    for a in range(2):
        for b in range(2):
            c = a * 2 + b
            lse = shift[b] + np.log(sumexp[:, c])
            cnt = np.bincount(mlab[b], minlength=1 << 14)[lab[a]].astype(np.float64)
            mlpp = (pos[:, c] / SUPCON_T - cnt * lse) / np.maximum(cnt, 1.0)
            losses[c] = -mlpp.mean()

    loss_contr = losses[0] + losses[3]        # (rgb,rgb) + (ir,ir)
    loss_cross = losses[1] + losses[2]        # (rgb,ir)  + (ir,rgb)
    return np.asarray([loss_contr, loss_cross], np.float32)


def run_device(in_maps, **kwargs):
    return run_bass_kernel_spmd(get_nc(), in_maps,
                                core_ids=list(range(NCORES)), **kwargs)


def kernel(inputs_rgb, inputs_ir, targets_rgb, targets_ir,
           features_rgb, features_ir,
           prototype_labels_rgb, prototype_labels_ir):
    in_maps, shift = make_in_maps(inputs_rgb, inputs_ir, targets_rgb,
                                  targets_ir, features_rgb, features_ir,
                                  prototype_labels_rgb, prototype_labels_ir)
    results = run_device(in_maps).results
    return combine(results, shift, targets_rgb, targets_ir,
                   prototype_labels_rgb, prototype_labels_ir)


# revision 7
# speedup vs baseline: 20502.2882x; 20502.2882x over previous
"""SupCon cluster-memory loss kernel for 8 TRN2 NeuronCores.

Problem: 4 SupCon losses (rgb/ir anchors x rgb/ir memory banks).
  logits = l2norm(x) @ mem.T / T   [256, 8192]
  loss   = -mean_i[ (sum_j mask*logprob) / max(sum_j mask, 1) ]

Sharding: memory banks split column-wise (N=8192 -> 1024 per core),
anchor batches replicated.  Each core computes, for its N-shard and all
4 (anchor, bank) combos:
  - sumexp[i] = sum_j exp(logits_ij - shift_b)   (ScalarE Exp w/ accum_out)
  - pos[i]    = sum_j [lab_i == mlab_j] * logits_ij  (VectorE ttr)
Host combines shards: LSE = shift + log(sum_cores sumexp), positives and
match counts, then the 4 scalar losses.

shift_b = max_row_norm(bank_b)/T is a per-bank upper bound on |logits|
(anchors are unit-norm), so exp args are <= 0: no overflow, and with
unit-norm banks the dynamic range is e^-28..1 -- no harmful underflow.
"""

from contextlib import ExitStack

import ml_dtypes
import numpy as np

BF16_NP = ml_dtypes.bfloat16

import concourse.bacc as bacc
import concourse.bass as bass
import concourse.mybir as mybir
import concourse.tile as tile
from concourse.bass_utils import run_bass_kernel_spmd

B = 256          # anchor batch per modality
N = 8192         # memory bank rows
D = 768          # feature dim
NCORES = 8
NS = N // NCORES     # 1024 bank rows per core
KT = D // 128        # 6 contraction tiles
MT = B // 128        # 2 anchor partition tiles
NT = NS // 512       # 2 psum free-dim tiles
SUPCON_T = 0.07

F32 = mybir.dt.float32
F32R = mybir.dt.float32r
BF16 = mybir.dt.bfloat16

_NC_CACHE = {}


def _build_nc():
    nc = bacc.Bacc("TRN2", target_bir_lowering=False, debug=False,
                   num_devices=NCORES)

    # Per-core DRAM inputs (host pre-transposed to K-major layouts).
    xT = nc.dram_tensor("xT", [2, KT, 128, B], BF16, kind="ExternalInput").ap()
    memT = nc.dram_tensor("memT", [2, KT, 128, NS], BF16, kind="ExternalInput").ap()
    lab_h = nc.dram_tensor("lab", [2, B], F32, kind="ExternalInput")
    mlab_h = nc.dram_tensor("mlab", [2, NS], F32, kind="ExternalInput")
    nshift_h = nc.dram_tensor("nshift", [2], F32, kind="ExternalInput")
    # Outputs: per anchor-row partial sums.  col = mt*8 + c*2 + nt,
    # combo c = a*2 + b.
    res_s = nc.dram_tensor("res_s", [128, 16], F32, kind="ExternalOutput").ap()
    res_p = nc.dram_tensor("res_p", [128, 16], F32, kind="ExternalOutput").ap()

    with tile.TileContext(nc) as tc, ExitStack() as ctx:
        const = ctx.enter_context(tc.tile_pool(name="const", bufs=1))
        wpool = ctx.enter_context(tc.tile_pool(name="wpool", bufs=1))
        mpool = ctx.enter_context(tc.tile_pool(name="mpool", bufs=1))
        eqpool = ctx.enter_context(tc.tile_pool(name="eqpool", bufs=1))
        psum = ctx.enter_context(tc.tile_pool(name="psum", bufs=8, space="PSUM"))
        ep = ctx.enter_context(tc.tile_pool(name="ep", bufs=3))
        outp = ctx.enter_context(tc.tile_pool(name="outp", bufs=1))

        rs = outp.tile([128, 16], F32, tag="rs", name="rs")
        rp = outp.tile([128, 16], F32, tag="rp", name="rp")

        # Anchor labels as [p, mt] and the per-bank -shift scalars.
        labt = []
        for a in range(2):
            t = const.tile([128, MT], F32, tag=f"lab{a}", name=f"lab{a}")
            nc.sync.dma_start(out=t, in_=lab_h.ap()[a].rearrange("(mt p) -> p mt", p=128))
            labt.append(t)
        shift_t = const.tile([128, 2], F32, tag="shift", name="shift")
        nc.sync.dma_start(out=shift_t,
                          in_=bass.AP(tensor=nshift_h, offset=0, ap=[[0, 128], [1, 2]]))

        # Bank prototype labels broadcast across partitions.
        mlabt = []
        for b in range(2):
            t = const.tile([128, NS], F32, tag=f"mlab{b}", name=f"mlab{b}")
            nc.sync.dma_start(out=t,
                              in_=bass.AP(tensor=mlab_h, offset=b * NS,
                                          ap=[[0, 128], [1, NS]]))
            mlabt.append(t)

        # Anchor features, K-major: [128, B] per (a, kt).
        xt = [[wpool.tile([128, B], BF16, tag=f"x{a}_{k}", name=f"x{a}_{k}") for k in range(KT)]
              for a in range(2)]
        for a in range(2):
            for k in range(KT):
                nc.sync.dma_start(out=xt[a][k], in_=xT[a, k])

        # Bank shards, K-major: [128, NS] per (b, kt); kt-major issue order so
        # the accumulation loop's operands arrive first.
        memt = [[None] * KT for _ in range(2)]
        for k in range(KT):
            for b in range(2):
                t = mpool.tile([128, NS], BF16, tag=f"m{b}_{k}", name=f"m{b}_{k}")
                nc.sync.dma_start(out=t, in_=memT[b, k])
                memt[b][k] = t

        # Label-match masks (GpSimd, frees VectorE for the reductions).
        eqt = {}
        for a in range(2):
            for b in range(2):
                for mt in range(MT):
                    t = eqpool.tile([128, NS], F32, tag=f"eq{a}{b}{mt}", name=f"eq{a}{b}{mt}")
                    nc.vector.tensor_scalar(out=t, in0=mlabt[b],
                                            scalar1=labt[a][:, mt:mt + 1],
                                            scalar2=None,
                                            op0=mybir.AluOpType.is_equal)
                    eqt[a, b, mt] = t

        for mt in range(MT):
            acc = {}
            for a in range(2):
                for b in range(2):
                    for nt in range(NT):
                        acc[a, b, nt] = psum.tile([128, 512], F32, tag="acc", name=f"acc{mt}_{a}{b}{nt}")
            for k in range(KT):
                for a in range(2):
                    lhsT = xt[a][k][:, mt * 128:(mt + 1) * 128]
                    for b in range(2):
                        for nt in range(NT):
                            nc.tensor.matmul(
                                acc[a, b, nt][:],
                                lhsT,
                                memt[b][k][:, nt * 512:(nt + 1) * 512],
                                start=(k == 0), stop=(k == KT - 1))
            for a in range(2):
                for b in range(2):
                    c = a * 2 + b
                    for nt in range(NT):
                        col = mt * 8 + c * 2 + nt
                        ex = ep.tile([128, 512], F32, tag="ex", name=f"ex{mt}_{c}{nt}")
                        nc.scalar.activation(
                            out=ex, in_=acc[a, b, nt][:],
                            func=mybir.ActivationFunctionType.Exp,
                            bias=shift_t[:, b:b + 1],
                            scale=1.0 / SUPCON_T,
                            accum_out=rs[:, col:col + 1])
                        # tensor_tensor_reduce faults on this runtime's DVE
                        # ucode; use mult + reduce instead.
                        msk = ep.tile([128, 512], F32, tag="msk", name=f"msk{mt}_{c}{nt}")
                        nc.vector.tensor_tensor(
                            out=msk,
                            in0=eqt[a, b, mt][:, nt * 512:(nt + 1) * 512],
                            in1=acc[a, b, nt][:],
                            op=mybir.AluOpType.mult)
                        nc.vector.tensor_reduce(
                            out=rp[:, col:col + 1], in_=msk,
                            axis=mybir.AxisListType.X,
                            op=mybir.AluOpType.add)

        nc.sync.dma_start(out=res_s, in_=rs)
        nc.sync.dma_start(out=res_p, in_=rp)

    nc.compile()
    return nc


def get_nc():
    if "nc" not in _NC_CACHE:
        _NC_CACHE["nc"] = _build_nc()
    return _NC_CACHE["nc"]


def _l2norm(x):
    n = np.linalg.norm(x, axis=-1, keepdims=True)
    return x / np.maximum(n, 1e-12)


def make_in_maps(inputs_rgb, inputs_ir, targets_rgb, targets_ir,
                 features_rgb, features_ir,
                 prototype_labels_rgb, prototype_labels_ir):
    x = [_l2norm(np.asarray(inputs_rgb, np.float32)),
         _l2norm(np.asarray(inputs_ir, np.float32))]
    feats = [np.asarray(features_rgb, np.float32),
             np.asarray(features_ir, np.float32)]
    lab = np.stack([np.asarray(targets_rgb), np.asarray(targets_ir)]
                   ).astype(np.float32)                       # [2, B]
    mlab_full = np.stack([np.asarray(prototype_labels_rgb),
                          np.asarray(prototype_labels_ir)]).astype(np.float32)

    xT = np.empty([2, KT, 128, B], BF16_NP)
    for a in range(2):
        xT[a] = x[a].T.reshape(KT, 128, B).astype(BF16_NP)

    bank_max = [float(np.sqrt((feats[b] ** 2).sum(axis=1).max()))
                for b in range(2)]
    shift = np.array([bank_max[0] / SUPCON_T, bank_max[1] / SUPCON_T],
                     np.float64)                              # added back on host
    nshift = (-shift).astype(np.float32)

    in_maps = []
    for c in range(NCORES):
        memT = np.empty([2, KT, 128, NS], BF16_NP)
        for b in range(2):
            memT[b] = np.ascontiguousarray(
                feats[b][c * NS:(c + 1) * NS, :].T).reshape(KT, 128, NS).astype(BF16_NP)
        in_maps.append({
            "xT": xT,
            "memT": memT,
            "lab": lab,
            "mlab": np.ascontiguousarray(mlab_full[:, c * NS:(c + 1) * NS]),
            "nshift": nshift,
        })
    return in_maps, shift


def combine(results, shift, targets_rgb, targets_ir,
            prototype_labels_rgb, prototype_labels_ir):
    # res arrays: [128, 16], col = mt*8 + c*2 + nt
    rs = np.stack([np.asarray(r["res_s"], np.float64) for r in results])
    rp = np.stack([np.asarray(r["res_p"], np.float64) for r in results])
    rs = rs.reshape(NCORES, 128, MT, 4, NT).sum(axis=(0, 4))  # [128, mt, c]
    rp = rp.reshape(NCORES, 128, MT, 4, NT).sum(axis=(0, 4))
    sumexp = rs.transpose(1, 0, 2).reshape(B, 4)              # anchor i = mt*128+p
    pos = rp.transpose(1, 0, 2).reshape(B, 4)

    lab = [np.asarray(targets_rgb).astype(np.int64),
           np.asarray(targets_ir).astype(np.int64)]
    mlab = [np.asarray(prototype_labels_rgb).astype(np.int64),
            np.asarray(prototype_labels_ir).astype(np.int64)]

    losses = np.zeros(4, np.float64)
    for a in range(2):
        for b in range(2):
            c = a * 2 + b
            lse = shift[b] + np.log(sumexp[:, c])
            cnt = np.bincount(mlab[b], minlength=1 << 14)[lab[a]].astype(np.float64)
            mlpp = (pos[:, c] / SUPCON_T - cnt * lse) / np.maximum(cnt, 1.0)
            losses[c] = -mlpp.mean()

    loss_contr = losses[0] + losses[3]        # (rgb,rgb) + (ir,ir)
    loss_cross = losses[1] + losses[2]        # (rgb,ir)  + (ir,rgb)
    return np.asarray([loss_contr, loss_cross], np.float32)


def run_device(in_maps, **kwargs):
    return run_bass_kernel_spmd(get_nc(), in_maps,
                                core_ids=list(range(NCORES)), **kwargs)


def kernel(inputs_rgb, inputs_ir, targets_rgb, targets_ir,
           features_rgb, features_ir,
           prototype_labels_rgb, prototype_labels_ir):
    in_maps, shift = make_in_maps(inputs_rgb, inputs_ir, targets_rgb,
                                  targets_ir, features_rgb, features_ir,
                                  prototype_labels_rgb, prototype_labels_ir)
    results = run_device(in_maps).results
    return combine(results, shift, targets_rgb, targets_ir,
                   prototype_labels_rgb, prototype_labels_ir)


# revision 9
# speedup vs baseline: 24883.7773x; 1.2137x over previous
"""SupCon cluster-memory loss kernel for 8 TRN2 NeuronCores.

Problem: 4 SupCon losses (rgb/ir anchors x rgb/ir memory banks).
  logits = l2norm(x) @ mem.T / T   [256, 8192]
  loss   = -mean_i[ (sum_j mask*log_prob) / max(sum_j mask, 1) ]

Sharding: memory banks split column-wise (N=8192 -> 1024 per core),
anchor batches replicated.  Each core computes, for its N-shard and all
4 (anchor, bank) combos, sumexp[i] = sum_j exp(logits_ij/T - shift_b)
via bf16 matmuls + ScalarE Exp with fused row-accumulate.

The positives term only touches the <=few memory rows whose prototype
label matches each anchor (exactly one for permutation labels).  The
host gathers those rows (index bookkeeping only, G_i = sum of matching
rows) and each of cores 0..3 computes one combo's positive dot products
pos_i = x_i . G_i on-device (VectorE mult+reduce over D).

Host combine: LSE_i = shift_b + log(sum_cores sumexp_i),
mlpp_i = (pos_i/T - cnt_i*LSE_i)/max(cnt_i,1), loss = -mean_i mlpp_i.

shift_b = max_row_norm(bank_b)/T bounds |logits| (anchors unit-norm):
exp args <= 0, no overflow; unit-norm banks keep the range e^-29..1.
"""

from contextlib import ExitStack

import ml_dtypes
import numpy as np

import concourse.bacc as bacc
import concourse.bass as bass
import concourse.mybir as mybir
import concourse.tile as tile
from concourse.bass_utils import run_bass_kernel_spmd

BF16_NP = ml_dtypes.bfloat16

B = 256          # anchor batch per modality
N = 8192         # memory bank rows
D = 768          # feature dim
NCORES = 8
NS = N // NCORES     # 1024 bank rows per core
KT = D // 128        # 6 contraction tiles
MT = B // 128        # 2 anchor partition tiles
NT = NS // 512       # 2 psum free-dim tiles
SUPCON_T = 0.07

F32 = mybir.dt.float32
BF16 = mybir.dt.bfloat16

_NC_CACHE = {}


def _build_nc():
    nc = bacc.Bacc("TRN2", target_bir_lowering=False, debug=False,
                   num_devices=NCORES)

    # Per-core DRAM inputs (host pre-transposed to K-major layouts).
    xT = nc.dram_tensor("xT", [2, KT, 128, B], BF16, kind="ExternalInput").ap()
    memT = nc.dram_tensor("memT", [2, KT, 128, NS], BF16, kind="ExternalInput").ap()
    nshift_h = nc.dram_tensor("nshift", [2], F32, kind="ExternalInput")
    # Positive pairs, anchor-major: this core's combo (cores 0-3; 4-7 get
    # zeros and their pos output is ignored).
    posx = nc.dram_tensor("posx", [MT, 128, D], BF16, kind="ExternalInput").ap()
    posg = nc.dram_tensor("posg", [MT, 128, D], BF16, kind="ExternalInput").ap()
    # Outputs: res_s col = mt*4 + c (combo c = a*2+b); res_p col = mt.
    res_s = nc.dram_tensor("res_s", [128, 8], F32, kind="ExternalOutput").ap()
    res_p = nc.dram_tensor("res_p", [128, MT], F32, kind="ExternalOutput").ap()

    with tile.TileContext(nc) as tc, ExitStack() as ctx:
        const = ctx.enter_context(tc.tile_pool(name="const", bufs=1))
        wpool = ctx.enter_context(tc.tile_pool(name="wpool", bufs=1))
        mpool = ctx.enter_context(tc.tile_pool(name="mpool", bufs=1))
        pospool = ctx.enter_context(tc.tile_pool(name="pospool", bufs=1))
        psum = ctx.enter_context(tc.tile_pool(name="psum", bufs=4, space="PSUM"))
        ep = ctx.enter_context(tc.tile_pool(name="ep", bufs=3))
        outp = ctx.enter_context(tc.tile_pool(name="outp", bufs=1))

        rs = outp.tile([128, 8], F32, tag="rs", name="rs")
        rp = outp.tile([128, MT], F32, tag="rp", name="rp")

        # First the tiles the PE needs earliest: kt-major bank shards on the
        # sync-engine DMA ring, anchor features on the scalar-engine ring.
        memt = [[None] * KT for _ in range(2)]
        for k in range(KT):
            for b in range(2):
                t = mpool.tile([128, NS], BF16, tag=f"m{b}_{k}", name=f"m{b}_{k}")
                nc.sync.dma_start(out=t, in_=memT[b, k])
                memt[b][k] = t

        xt = [[wpool.tile([128, B], BF16, tag=f"x{a}_{k}", name=f"x{a}_{k}")
               for k in range(KT)] for a in range(2)]
        for k in range(KT):
            for a in range(2):
                nc.scalar.dma_start(out=xt[a][k], in_=xT[a, k])

        shift_t = const.tile([128, 2], F32, tag="shift", name="shift")
        nc.scalar.dma_start(out=shift_t,
                            in_=bass.AP(tensor=nshift_h, offset=0,
                                        ap=[[0, 128], [1, 2]]))

        pxt, pgt = [], []
        for mt in range(MT):
            tx = pospool.tile([128, D], BF16, tag=f"px{mt}", name=f"px{mt}")
            tg = pospool.tile([128, D], BF16, tag=f"pg{mt}", name=f"pg{mt}")
            nc.scalar.dma_start(out=tx, in_=posx[mt])
            nc.scalar.dma_start(out=tg, in_=posg[mt])
            pxt.append(tx)
            pgt.append(tg)

        # Positive dot products (VectorE; tiny).
        for mt in range(MT):
            pm = ep.tile([128, D], F32, tag="pm", name=f"pm{mt}")
            nc.vector.tensor_tensor(out=pm, in0=pxt[mt], in1=pgt[mt],
                                    op=mybir.AluOpType.mult)
            nc.vector.tensor_reduce(out=rp[:, mt:mt + 1], in_=pm,
                                    axis=mybir.AxisListType.X,
                                    op=mybir.AluOpType.add)

        for mt in range(MT):
            acc = {}
            for a in range(2):
                for b in range(2):
                    acc[a, b] = psum.tile([128, NS], F32, tag="acc",
                                          name=f"acc{mt}_{a}{b}")
            for k in range(KT):
                for a in range(2):
                    lhsT = xt[a][k][:, mt * 128:(mt + 1) * 128]
                    for b in range(2):
                        for nt in range(NT):
                            nc.tensor.matmul(
                                acc[a, b][:, nt * 512:(nt + 1) * 512],
                                lhsT,
                                memt[b][k][:, nt * 512:(nt + 1) * 512],
                                start=(k == 0), stop=(k == KT - 1))
            for a in range(2):
                for b in range(2):
                    c = a * 2 + b
                    ex = ep.tile([128, NS], F32, tag="ex", name=f"ex{mt}_{c}")
                    nc.scalar.activation(
                        out=ex, in_=acc[a, b][:],
                        func=mybir.ActivationFunctionType.Exp,
                        bias=shift_t[:, b:b + 1],
                        scale=1.0 / SUPCON_T,
                        accum_out=rs[:, mt * 4 + c:mt * 4 + c + 1])

        nc.sync.dma_start(out=res_s, in_=rs)
        nc.sync.dma_start(out=res_p, in_=rp)

    nc.compile()
    return nc


def get_nc():
    if "nc" not in _NC_CACHE:
        _NC_CACHE["nc"] = _build_nc()
    return _NC_CACHE["nc"]


def _l2norm(x):
    n = np.linalg.norm(x, axis=-1, keepdims=True)
    return x / np.maximum(n, 1e-12)


def _gather_positives(feats_b, lab_a, mlab_b):
    """G[i] = sum of bank rows whose prototype label == lab_a[i].

    Pure index bookkeeping for permutation labels (single match); falls
    back to a scatter-add for general labels."""
    G = np.zeros((B, D), np.float32)
    if np.unique(mlab_b).size == mlab_b.size:
        inv = np.full(1 << 14, -1, np.int64)
        inv[mlab_b] = np.arange(mlab_b.size)
        idx = inv[np.clip(lab_a, 0, (1 << 14) - 1)]
        valid = idx >= 0
        G[valid] = feats_b[idx[valid]]
    else:
        by_label = np.zeros((1 << 14, D), np.float32)
        np.add.at(by_label, mlab_b, feats_b)
        G[:] = by_label[np.clip(lab_a, 0, (1 << 14) - 1)]
    return G


def make_in_maps(inputs_rgb, inputs_ir, targets_rgb, targets_ir,
                 features_rgb, features_ir,
                 prototype_labels_rgb, prototype_labels_ir):
    x = [_l2norm(np.asarray(inputs_rgb, np.float32)),
         _l2norm(np.asarray(inputs_ir, np.float32))]
    feats = [np.asarray(features_rgb, np.float32),
             np.asarray(features_ir, np.float32)]
    lab = [np.asarray(targets_rgb).astype(np.int64),
           np.asarray(targets_ir).astype(np.int64)]
    mlab = [np.asarray(prototype_labels_rgb).astype(np.int64),
            np.asarray(prototype_labels_ir).astype(np.int64)]

    xT = np.empty([2, KT, 128, B], BF16_NP)
    for a in range(2):
        xT[a] = x[a].T.reshape(KT, 128, B).astype(BF16_NP)

    bank_max = [float(np.sqrt((feats[b] ** 2).sum(axis=1).max()))
                for b in range(2)]
    shift = np.array([bank_max[0] / SUPCON_T, bank_max[1] / SUPCON_T],
                     np.float64)
    nshift = (-shift).astype(np.float32)

    zeros_pos = np.zeros([MT, 128, D], BF16_NP)
    in_maps = []
    for c in range(NCORES):
        memT = np.empty([2, KT, 128, NS], BF16_NP)
        for b in range(2):
            memT[b] = np.ascontiguousarray(
                feats[b][c * NS:(c + 1) * NS, :].T).reshape(KT, 128, NS).astype(BF16_NP)
        if c < 4:
            a, b = c // 2, c % 2
            G = _gather_positives(feats[b], lab[a], mlab[b])
            posx = x[a].reshape(MT, 128, D).astype(BF16_NP)
            posg = G.reshape(MT, 128, D).astype(BF16_NP)
        else:
            posx = posg = zeros_pos
        in_maps.append({
            "xT": xT,
            "memT": memT,
            "nshift": nshift,
            "posx": posx,
            "posg": posg,
        })
    return in_maps, shift


def combine(results, shift, targets_rgb, targets_ir,
            prototype_labels_rgb, prototype_labels_ir):
    rs = np.stack([np.asarray(r["res_s"], np.float64) for r in results])
    rs = rs.reshape(NCORES, 128, MT, 4).sum(axis=0)           # [128, mt, c]
    sumexp = rs.transpose(1, 0, 2).reshape(B, 4)              # i = mt*128+p

    lab = [np.asarray(targets_rgb).astype(np.int64),
           np.asarray(targets_ir).astype(np.int64)]
    mlab = [np.asarray(prototype_labels_rgb).astype(np.int64),
            np.asarray(prototype_labels_ir).astype(np.int64)]

    losses = np.zeros(4, np.float64)
    for a in range(2):
        for b in range(2):
            c = a * 2 + b
            pos = np.asarray(results[c]["res_p"], np.float64).T.reshape(B)
            lse = shift[b] + np.log(sumexp[:, c])
            cnt = np.bincount(mlab[b], minlength=1 << 14)[
                np.clip(lab[a], 0, (1 << 14) - 1)].astype(np.float64)
            mlpp = (pos / SUPCON_T - cnt * lse) / np.maximum(cnt, 1.0)
            losses[c] = -mlpp.mean()

    loss_contr = losses[0] + losses[3]        # (rgb,rgb) + (ir,ir)
    loss_cross = losses[1] + losses[2]        # (rgb,ir)  + (ir,rgb)
    return np.asarray([loss_contr, loss_cross], np.float32)


def run_device(in_maps, **kwargs):
    return run_bass_kernel_spmd(get_nc(), in_maps,
                                core_ids=list(range(NCORES)), **kwargs)


def kernel(inputs_rgb, inputs_ir, targets_rgb, targets_ir,
           features_rgb, features_ir,
           prototype_labels_rgb, prototype_labels_ir):
    in_maps, shift = make_in_maps(inputs_rgb, inputs_ir, targets_rgb,
                                  targets_ir, features_rgb, features_ir,
                                  prototype_labels_rgb, prototype_labels_ir)
    results = run_device(in_maps).results
    return combine(results, shift, targets_rgb, targets_ir,
                   prototype_labels_rgb, prototype_labels_ir)
